# revision 1
# baseline (speedup 1.0000x reference)
"""LightGCN 3-layer propagation + batch dot on 8 Trainium2 NeuronCores.

Transfer-diet rewrite of the original one-hot-matmul kernel. The per-call
device invocation previously shipped ~590MB over the host link (full node
table replicated to all 8 cores, 8x-replicated gather indices, f32
metadata, 38.5MB output + donated zeros); this version ships ~94MB:
  (a) node table AllGathered on device from per-core f32 shards,
  (b) gather indices sent un-replicated [16, C*8] and replicated
      16->128 partitions by 8 on-device DMAs per strip,
  (c) per-edge one-hot row ids sent as uint8, converted on device,
  (d) the batch dot epilogue computed on device (AllGather acc ->
      banked dma_gather of the 8192 batch rows -> dma_scatter_add into
      slot order -> elementwise dot) so D2H is 16KB per core.
All arithmetic stays f32 (fp16 val/product variants pass but with only
1.4x margin against the 2e-2 rel-err gate; f32 gives 360x).

Per layer (unchanged from the original scheme): row-partitioned nodes,
per (128-row tile, col-bank) fixed-capacity edge chunks, dma_gather of
source embeddings (col-sorted within chunks for DRAM locality), val
multiply, one-hot f32 matmuls segment-summing into f32 PSUM.
DGE note: num_idxs_reg must equal the count of non-negative indices, so
all index streams are padded with valid indices (row 0 for gathers, dump
rows past slot 8191 for the epilogue scatter).
"""
import numpy as np

N_USERS = 100000
N_ITEMS = 50000
N = N_USERS + N_ITEMS        # 150000
D = 64
NCORES = 8
N_PAD = 150528               # 8 * 18816
R = N_PAD // NCORES          # 18816 rows per core
T = R // 128                 # 147 row-tiles per core
STRIP = 21                   # tiles per metadata strip (147 = 7*21)
NB = 5                       # 32768-row col banks
BANK_BASE = (0, 32768, 65536, 98304, 131072)
BATCH = 4096
# epilogue per-bank gather capacities (rows 0..150527, bank = row>>15)
EPC_MIN = (1536, 1536, 1536, 3072, 1792)

_compiled = {}


def _preprocess(edge_row, edge_col, edge_val):
    """Sort/pad edges into per-core fixed-capacity (tile, bank) chunks.

    Returns (L, idxc [8,16,C*8] i16, valv [8,128,C] f16, rlv [8,128,C] i16)
    where L = per-bank chunk capacities (in 128-edge units) and C = T*sum(L).
    Within each chunk edges are sorted by column index for gather locality.
    """
    er = np.asarray(edge_row).astype(np.int64)
    ec = np.asarray(edge_col).astype(np.int64)
    ev = np.asarray(edge_val).astype(np.float32)

    owner = er // R
    rrem = er - owner * R
    tloc = rrem >> 7
    rl = (rrem & 127).astype(np.uint8)
    bank = ec >> 15
    cidx = (ec & 32767).astype(np.int16)

    seg = (owner * T + tloc) * NB + bank
    nseg = NCORES * T * NB
    counts = np.bincount(seg, minlength=nseg)
    cmax = counts.reshape(NCORES, T, NB).max(axis=(0, 1))
    L = tuple(int(-(-int(c) // 128)) for c in cmax)     # ceil/128
    LT = sum(L)
    C = T * LT

    order = np.argsort(seg * 32768 + cidx.astype(np.int64))
    sseg = seg[order]
    starts = np.concatenate([[0], np.cumsum(counts)[:-1]])
    rank = np.arange(len(order)) - starts[sseg]

    segL = np.concatenate([[0], np.cumsum(L)[:-1]])
    core_o, tloc_o, bank_o = owner[order], tloc[order], bank[order]
    pos = (tloc_o * LT + segL[bank_o]) * 128 + rank     # within-core edge slot

    E_cap = C * 128
    # pad slots: idx 0 (valid row, gathers garbage), val 0, rl 0 -> adds 0
    cidx_a = np.zeros((NCORES, E_cap), dtype=np.int16)
    val_a = np.zeros((NCORES, E_cap), dtype=np.float32)
    rl_a = np.zeros((NCORES, E_cap), dtype=np.uint8)
    cidx_a[core_o, pos] = cidx[order]
    val_a[core_o, pos] = ev[order]
    rl_a[core_o, pos] = rl[order]

    idxc = cidx_a.reshape(NCORES, C * 8, 16).transpose(0, 2, 1).copy()  # [8,16,C*8]
    valv = val_a.reshape(NCORES, C, 128).transpose(0, 2, 1).copy()      # [8,128,C]
    rlv = rl_a.reshape(NCORES, C, 128).transpose(0, 2, 1).copy()        # [8,128,C]
    return L, idxc, valv, rlv


def _ep_meta(users, items):
    """Banked gather + slot-scatter indices for the batch-dot epilogue.

    Slot s in [0,4096) is user s; slot 4096+s is item s. Returns
    (caps, gidx [128, G/16] i16, sidx [128, G/16] i16) with G = sum(caps).
    All indices are valid (num_idxs_reg must equal the count of
    non-negative indices): gather pads hit row 0 of the bank, scatter
    pads land in the 128 dump rows past the 8192 real slots.
    """
    rows = np.concatenate([
        np.asarray(users).astype(np.int64),
        N_USERS + np.asarray(items).astype(np.int64),
    ])
    bank = rows >> 15
    cidx = (rows & 32767).astype(np.int16)
    order = np.argsort(bank, kind="stable")
    counts = np.bincount(bank, minlength=NB)
    caps = tuple(
        int(max(EPC_MIN[b], -(-int(counts[b]) // 128) * 128)) for b in range(NB)
    )
    G = sum(caps)
    gidx = np.zeros(G, dtype=np.int16)
    sidx = (2 * BATCH + (np.arange(G) & 127)).astype(np.int16)  # dump rows
    off = 0
    p = 0
    for b in range(NB):
        cnt = int(counts[b])
        sel = order[p:p + cnt]
        p += cnt
        gidx[off:off + cnt] = cidx[sel]
        sidx[off:off + cnt] = sel.astype(np.int16)      # slot id
        off += caps[b]
    gw = np.tile(gidx.reshape(G // 16, 16).T, (8, 1)).copy()  # [128, G/16]
    sw = np.tile(sidx.reshape(G // 16, 16).T, (8, 1)).copy()
    return caps, gw, sw


def _build(L, EPC):
    import concourse.bacc as bacc
    import concourse.mybir as mybir
    import concourse.tile as tile
    from concourse.library_config import mlp

    LT = sum(L)
    C = T * LT
    G = sum(EPC)
    f32 = mybir.dt.float32
    i16 = mybir.dt.int16
    u8 = mybir.dt.uint8

    EP, IX, CC = 3, 1, 7   # production path (debug bisect knobs, hardcoded)

    nc = bacc.Bacc("TRN2", target_bir_lowering=False, debug=False,
                   num_devices=NCORES)
    e0 = nc.dram_tensor("e0", [R, D], f32, kind="ExternalInput")
    idxc = nc.dram_tensor("idxc", [16 if IX else 128, C * 8], i16,
                          kind="ExternalInput")
    valv = nc.dram_tensor("valv", [128, C], f32, kind="ExternalInput")
    rlv = nc.dram_tensor("rlv", [128, C], u8, kind="ExternalInput")
    egi = nc.dram_tensor("egi", [128, G // 16], i16, kind="ExternalInput")
    esi = nc.dram_tensor("esi", [128, G // 16], i16, kind="ExternalInput")
    outg = nc.dram_tensor("outg", [BATCH], f32, kind="ExternalOutput")

    segc = [0]
    for x in L[:-1]:
        segc.append(segc[-1] + x)
    RG = [list(range(NCORES))]

    with tile.TileContext(nc, num_cores=NCORES) as tc:
        with tc.tile_pool(name="const", bufs=1) as constp, \
             tc.tile_pool(name="accp", bufs=1) as accp, \
             tc.tile_pool(name="psum", bufs=4, space="PSUM") as psp, \
             tc.tile_pool(name="dram", bufs=1, space="DRAM") as dram:
            nc.gpsimd.load_library(mlp)
            iota = constp.tile([128, 1, 128], i16)
            nc.gpsimd.iota(iota[:, 0, :], pattern=[[1, 128]], base=0,
                           channel_multiplier=0)
            acc = accp.tile([128, T * D], f32)
            nc.sync.dma_start(out=acc[:].rearrange("p (t d) -> p t d", d=D),
                              in_=e0[:].rearrange("(t p) d -> p t d", p=128))

            tb0 = dram.tile([N_PAD, D], f32, tag="tb0", addr_space="Shared")
            tb1 = dram.tile([N_PAD, D], f32, tag="tb1", addr_space="Shared")
            tb2 = dram.tile([N_PAD, D], f32, tag="tb2", addr_space="Shared")
            sh1 = dram.tile([R, D], f32, tag="sh1")
            sh2 = dram.tile([R, D], f32, tag="sh2")
            accd = dram.tile([R, D], f32, tag="accd")
            accf = dram.tile([N_PAD, D], f32, tag="accf", addr_space="Shared")
            ues = dram.tile([2 * BATCH + 128, D], f32, tag="ues")  # +dump rows

            # full node table from per-core shards (replaces 308MB H2D).
            # Collectives cannot read IO tensors -> stage through sh0.
            sh0 = dram.tile([R, D], f32, tag="sh0")
            nc.sync.dma_start(out=sh0[:], in_=e0[:])
            if CC & 1:
                nc.gpsimd.collective_compute("AllGather", mybir.AluOpType.bypass,
                                             replica_groups=RG,
                                             ins=[sh0[:]], outs=[tb0[:]])
            else:
                nc.gpsimd.dma_start(out=tb0[:][:R], in_=sh0[:])
            # zero the epilogue scatter target early (overlaps with layers)
            zt = constp.tile([128, 2 * BATCH // 128, D], f32)
            nc.vector.memset(zt[:], 0)
            nc.sync.dma_start(
                out=ues[:][0:2 * BATCH, :].rearrange("(c p) d -> p c d", p=128),
                in_=zt[:])
            egs = constp.tile([128, G // 16], i16)
            nc.sync.dma_start(out=egs[:], in_=egi[:])
            ess = constp.tile([128, G // 16], i16)
            nc.sync.dma_start(out=ess[:], in_=esi[:])

            tables = [tb0, tb1, tb2]
            shards = [sh1, sh2]

            with tc.tile_pool(name="meta", bufs=2) as metap, \
                 tc.tile_pool(name="gp", bufs=3) as gp, \
                 tc.tile_pool(name="sp", bufs=2) as sp, \
                 tc.tile_pool(name="ob", bufs=4) as obp:
                for layer in range(3):
                    src = tables[layer][:]
                    for s in range(T // STRIP):
                        cols = STRIP * LT
                        c0s = s * cols
                        ixs = metap.tile([128, cols * 8], i16, tag="ixs")
                        if IX:
                            for a in range(8):
                                nc.sync.dma_start(
                                    out=ixs[16 * a:16 * (a + 1), :],
                                    in_=idxc[:, c0s * 8:(c0s + cols) * 8])
                        else:
                            nc.sync.dma_start(
                                out=ixs[:],
                                in_=idxc[:, c0s * 8:(c0s + cols) * 8])
                        vls = metap.tile([128, cols], f32, tag="vls")
                        nc.sync.dma_start(out=vls[:], in_=valv[:, c0s:c0s + cols])
                        rls8 = metap.tile([128, cols], u8, tag="rls8")
                        nc.sync.dma_start(out=rls8[:], in_=rlv[:, c0s:c0s + cols])
                        rls = metap.tile([128, cols], i16, tag="rls")
                        nc.scalar.copy(out=rls[:], in_=rls8[:])
                        for tt in range(STRIP):
                            t = s * STRIP + tt
                            ps = psp.tile([128, D], f32)
                            S = sp.tile([128, LT, 128], f32, tag="S")
                            nc.vector.tensor_tensor(
                                out=S[:],
                                in0=rls[:, tt * LT:(tt + 1) * LT].to_broadcast([128, LT, 128]),
                                in1=iota[:].to_broadcast([128, LT, 128]),
                                op=mybir.AluOpType.is_equal)
                            for b in range(NB):
                                Lb = L[b]
                                if Lb == 0:
                                    continue
                                g = gp.tile([128, Lb, D], f32, tag=f"g{b}")
                                ib = (tt * LT + segc[b]) * 8
                                nc.gpsimd.dma_gather(
                                    g[:], src[BANK_BASE[b]:, :],
                                    ixs[:, ib:ib + Lb * 8], Lb * 128, Lb * 128,
                                    D, single_packet=False)
                                vb = tt * LT + segc[b]
                                nc.vector.tensor_tensor(
                                    out=g[:],
                                    in0=vls[:, vb:vb + Lb].to_broadcast([128, Lb, D]),
                                    in1=g[:],
                                    op=mybir.AluOpType.mult)
                                for k in range(Lb):
                                    kk = segc[b] + k
                                    nc.tensor.matmul(
                                        out=ps[:], lhsT=S[:, kk, :], rhs=g[:, k, :],
                                        start=(kk == 0), stop=(kk == LT - 1))
                            nc.vector.tensor_add(out=acc[:, t * D:(t + 1) * D],
                                                 in0=acc[:, t * D:(t + 1) * D],
                                                 in1=ps[:])
                            if layer < 2:
                                ob = obp.tile([128, D], f32, tag="ob")
                                nc.scalar.copy(out=ob[:], in_=ps[:])
                                nc.sync.dma_start(
                                    out=shards[layer][:].rearrange(
                                        "(t p) d -> p t d", p=128)[:, t, :],
                                    in_=ob[:])
                        del ixs, vls, rls8, rls
                    if layer < 2:
                        if CC & 2:
                            nc.gpsimd.collective_compute(
                                "AllGather", mybir.AluOpType.bypass,
                                replica_groups=RG,
                                ins=[shards[layer][:]], outs=[tables[layer + 1][:]])
                        else:
                            nc.gpsimd.dma_start(out=tables[layer + 1][:][:R],
                                                in_=shards[layer][:])

            # ---- epilogue: gamma[s] = (acc[u_s] . acc[N_USERS+i_s]) / 16 ----
            nc.sync.dma_start(out=accd[:].rearrange("(t p) d -> p t d", p=128),
                              in_=acc[:].rearrange("p (t d) -> p t d", d=D))
            if CC & 4:
                nc.gpsimd.collective_compute("AllGather", mybir.AluOpType.bypass,
                                             replica_groups=RG,
                                             ins=[accd[:]], outs=[accf[:]])
            else:
                nc.gpsimd.dma_start(out=accf[:][:R], in_=accd[:])
            with tc.tile_pool(name="ep", bufs=1) as epp:
                half = BATCH // 128
                if EP >= 0:
                    goff = 0
                    for b in range(NB):
                        cap = EPC[b]
                        if cap == 0:
                            continue
                        gb = epp.tile([128, cap // 128, D], f32, tag=f"eg{b}")
                        if EP >= 1:
                            nc.gpsimd.dma_gather(
                                gb[:], accf[:][BANK_BASE[b]:, :],
                                egs[:, goff // 16:(goff + cap) // 16], cap, cap,
                                D, single_packet=False)
                        if EP >= 2:
                            nc.gpsimd.dma_scatter_add(
                                ues[:], gb[:],
                                ess[:, goff // 16:(goff + cap) // 16], cap, cap,
                                D, single_packet=False)
                        goff += cap
                    ue_sb = epp.tile([128, 2 * BATCH // 128, D], f32)
                    nc.sync.dma_start(
                        out=ue_sb[:],
                        in_=ues[:][0:2 * BATCH, :].rearrange("(c p) d -> p c d", p=128))
                    prod = epp.tile([128, half, D], f32)
                    nc.vector.tensor_tensor(out=prod[:], in0=ue_sb[:, 0:half, :],
                                            in1=ue_sb[:, half:2 * half, :],
                                            op=mybir.AluOpType.mult)
                    gm = epp.tile([128, half], f32)
                    nc.vector.tensor_reduce(out=gm[:], in_=prod[:],
                                            axis=mybir.AxisListType.X,
                                            op=mybir.AluOpType.add)
                    gms = epp.tile([128, half], f32)
                    nc.vector.tensor_scalar_mul(out=gms[:], in0=gm[:],
                                                scalar1=1.0 / 16.0)
                else:
                    gms = epp.tile([128, half], f32)
                    nc.vector.memset(gms[:], 0)
                nc.sync.dma_start(out=outg[:].rearrange("(c p) -> p c", p=128),
                                  in_=gms[:])
    nc.compile()
    return nc


def kernel(user_emb, item_emb, edge_row, edge_col, edge_val, users, items):
    from concourse.bass_utils import run_bass_kernel_spmd

    e0_full = np.zeros((N_PAD, D), dtype=np.float32)
    e0_full[:N_USERS] = np.asarray(user_emb, dtype=np.float32)
    e0_full[N_USERS:N] = np.asarray(item_emb, dtype=np.float32)

    L, idxc, valv, rlv = _preprocess(edge_row, edge_col, edge_val)
    caps, gw, sw = _ep_meta(users, items)
    key = (L, caps)
    if key not in _compiled:
        _compiled[key] = _build(L, caps)
    nc = _compiled[key]

    in_maps = []
    for c in range(NCORES):
        in_maps.append({
            "e0": e0_full[c * R:(c + 1) * R],
            "idxc": idxc[c],
            "valv": valv[c],
            "rlv": rlv[c],
            "egi": gw,
            "esi": sw,
        })
    res = run_bass_kernel_spmd(nc, in_maps, core_ids=list(range(NCORES)))
    return np.asarray(res.results[0]["outg"], dtype=np.float32)



# revision 2
# speedup vs baseline: 2.1508x; 2.1508x over previous
"""LightGCN 3-layer propagation + batch dot on 8 Trainium2 NeuronCores.

Transfer-diet + dispatch-diet revision. Per-call device invocation cost
under the axon tunnel decomposes as: MLIR lower (~0.3s), BIR->NEFF walrus
recompile (~2.0s without caching), H2D at ~60-100MB/s, device exec
(~0.5s), D2H (tiny). This version attacks the first three:
  (a) JAX persistent compilation cache enabled (module import time) so
      repeat calls skip the ~2s walrus/neuronxcc recompile entirely,
  (b) e0 shipped as int24 fixed-point (int16 hi plane + uint8 lo plane,
      scale 2^-23; reconstruction on device is exact in f32) - 38.6MB ->
      28.9MB, max-rel-err vs f32 reference 7.6e-4 (26x margin, measured
      host-side on the real data; fp16 e0 FAILS at 2.1e-1),
  (c) edge vals shipped as uint16 fixed-point (scale 2^-22, exact
      reconstruct; max val 0.01*2^22 = 41943 < 65536) - 15.4MB -> 7.7MB,
  (d) epilogue gather/scatter indices shipped un-replicated [16, G/16]
      and replicated 16->128 on device like the edge indices.
All device arithmetic stays f32 (gather tables, products, PSUM).

Per layer (unchanged): row-partitioned nodes, per (128-row tile,
col-bank) fixed-capacity edge chunks, dma_gather of source embeddings
(col-sorted within chunks), val multiply, one-hot f32 matmuls
segment-summing into f32 PSUM, AllGather of the next layer table.
DGE note: num_idxs_reg must equal the count of non-negative indices, so
all index streams are padded with valid indices (row 0 for gathers, dump
rows past slot 8191 for the epilogue scatter).
"""
import numpy as np

N_USERS = 100000
N_ITEMS = 50000
N = N_USERS + N_ITEMS        # 150000
D = 64
NCORES = 8
N_PAD = 150528               # 8 * 18816
R = N_PAD // NCORES          # 18816 rows per core
T = R // 128                 # 147 row-tiles per core
STRIP = 21                   # tiles per metadata strip (147 = 7*21)
NB = 5                       # 32768-row col banks
BANK_BASE = (0, 32768, 65536, 98304, 131072)
BATCH = 4096
# epilogue per-bank gather capacities (rows 0..150527, bank = row>>15)
EPC_MIN = (1536, 1536, 1536, 3072, 1792)
E0_SCALE = float(2.0 ** -23)  # e0 int24 fixed-point scale
EV_SCALE = float(2.0 ** -22)  # edge_val uint16 fixed-point scale

_compiled = {}
_cache_configured = False


def _configure_jax_cache():
    """Persistent compilation cache: repeat calls (and repeat processes)
    skip the ~2s BIR->NEFF walrus recompile that run_bass_via_pjrt's
    fresh-jit-per-call path otherwise re-runs every invocation."""
    global _cache_configured
    if _cache_configured:
        return
    import jax
    jax.config.update("jax_compilation_cache_dir", "/tmp/jax_bass_cache")
    jax.config.update("jax_persistent_cache_min_compile_time_secs", 0.0)
    jax.config.update("jax_persistent_cache_min_entry_size_bytes", 0)
    _cache_configured = True


def _preprocess(edge_row, edge_col, edge_val):
    """Sort/pad edges into per-core fixed-capacity (tile, bank) chunks.

    Returns (L, idxc [8,16,C*8] i16, valv [8,128,C] u16, rlv [8,128,C] u8)
    where L = per-bank chunk capacities (in 128-edge units) and C = T*sum(L).
    Within each chunk edges are sorted by column index for gather locality.
    Vals are uint16 fixed-point (v * 2^22 rounded; exact f32 reconstruct).
    """
    er = np.asarray(edge_row).astype(np.int64)
    ec = np.asarray(edge_col).astype(np.int64)
    ev = np.asarray(edge_val).astype(np.float64)

    owner = er // R
    rrem = er - owner * R
    tloc = rrem >> 7
    rl = (rrem & 127).astype(np.uint8)
    bank = ec >> 15
    cidx = (ec & 32767).astype(np.int16)

    seg = (owner * T + tloc) * NB + bank
    nseg = NCORES * T * NB
    counts = np.bincount(seg, minlength=nseg)
    cmax = counts.reshape(NCORES, T, NB).max(axis=(0, 1))
    L = tuple(int(-(-int(c) // 128)) for c in cmax)     # ceil/128
    LT = sum(L)
    C = T * LT

    order = np.argsort(seg * 32768 + cidx.astype(np.int64))
    sseg = seg[order]
    starts = np.concatenate([[0], np.cumsum(counts)[:-1]])
    rank = np.arange(len(order)) - starts[sseg]

    segL = np.concatenate([[0], np.cumsum(L)[:-1]])
    core_o, tloc_o, bank_o = owner[order], tloc[order], bank[order]
    pos = (tloc_o * LT + segL[bank_o]) * 128 + rank     # within-core edge slot

    evq = np.clip(np.round(ev / EV_SCALE), 0, 65535).astype(np.uint16)

    E_cap = C * 128
    # pad slots: idx 0 (valid row, gathers garbage), val 0, rl 0 -> adds 0
    cidx_a = np.zeros((NCORES, E_cap), dtype=np.int16)
    val_a = np.zeros((NCORES, E_cap), dtype=np.uint16)
    rl_a = np.zeros((NCORES, E_cap), dtype=np.uint8)
    cidx_a[core_o, pos] = cidx[order]
    val_a[core_o, pos] = evq[order]
    rl_a[core_o, pos] = rl[order]

    idxc = cidx_a.reshape(NCORES, C * 8, 16).transpose(0, 2, 1).copy()  # [8,16,C*8]
    valv = val_a.reshape(NCORES, C, 128).transpose(0, 2, 1).copy()      # [8,128,C]
    rlv = rl_a.reshape(NCORES, C, 128).transpose(0, 2, 1).copy()        # [8,128,C]
    return L, idxc, valv, rlv


def _ep_meta(users, items):
    """Banked gather + slot-scatter indices for the batch-dot epilogue.

    Slot s in [0,4096) is user s; slot 4096+s is item s. Returns
    (caps, gidx [16, G/16] i16, sidx [16, G/16] i16) with G = sum(caps);
    replicated 16->128 partitions on device. All indices are valid
    (num_idxs_reg must equal the count of non-negative indices): gather
    pads hit row 0 of the bank, scatter pads land in the 128 dump rows
    past the 8192 real slots.
    """
    rows = np.concatenate([
        np.asarray(users).astype(np.int64),
        N_USERS + np.asarray(items).astype(np.int64),
    ])
    bank = rows >> 15
    cidx = (rows & 32767).astype(np.int16)
    order = np.argsort(bank, kind="stable")
    counts = np.bincount(bank, minlength=NB)
    caps = tuple(
        int(max(EPC_MIN[b], -(-int(counts[b]) // 128) * 128)) for b in range(NB)
    )
    G = sum(caps)
    gidx = np.zeros(G, dtype=np.int16)
    sidx = (2 * BATCH + (np.arange(G) & 127)).astype(np.int16)  # dump rows
    off = 0
    p = 0
    for b in range(NB):
        cnt = int(counts[b])
        sel = order[p:p + cnt]
        p += cnt
        gidx[off:off + cnt] = cidx[sel]
        sidx[off:off + cnt] = sel.astype(np.int16)      # slot id
        off += caps[b]
    gw = gidx.reshape(G // 16, 16).T.copy()   # [16, G/16]
    sw = sidx.reshape(G // 16, 16).T.copy()
    return caps, gw, sw


def _build(L, EPC):
    import concourse.bacc as bacc
    import concourse.mybir as mybir
    import concourse.tile as tile
    from concourse.library_config import mlp

    LT = sum(L)
    C = T * LT
    G = sum(EPC)
    f32 = mybir.dt.float32
    i16 = mybir.dt.int16
    u16 = mybir.dt.uint16
    u8 = mybir.dt.uint8

    nc = bacc.Bacc("TRN2", target_bir_lowering=False, debug=False,
                   num_devices=NCORES)
    e0hi = nc.dram_tensor("e0hi", [R, D], i16, kind="ExternalInput")
    e0lo = nc.dram_tensor("e0lo", [R, D], u8, kind="ExternalInput")
    idxc = nc.dram_tensor("idxc", [16, C * 8], i16, kind="ExternalInput")
    valv = nc.dram_tensor("valv", [128, C], u16, kind="ExternalInput")
    rlv = nc.dram_tensor("rlv", [128, C], u8, kind="ExternalInput")
    egi = nc.dram_tensor("egi", [16, G // 16], i16, kind="ExternalInput")
    esi = nc.dram_tensor("esi", [16, G // 16], i16, kind="ExternalInput")
    outg = nc.dram_tensor("outg", [BATCH], f32, kind="ExternalOutput")

    segc = [0]
    for x in L[:-1]:
        segc.append(segc[-1] + x)
    RG = [list(range(NCORES))]

    with tile.TileContext(nc, num_cores=NCORES) as tc:
        with tc.tile_pool(name="const", bufs=1) as constp, \
             tc.tile_pool(name="accp", bufs=1) as accp, \
             tc.tile_pool(name="psum", bufs=4, space="PSUM") as psp, \
             tc.tile_pool(name="dram", bufs=1, space="DRAM") as dram:
            nc.gpsimd.load_library(mlp)
            iota = constp.tile([128, 1, 128], i16)
            nc.gpsimd.iota(iota[:, 0, :], pattern=[[1, 128]], base=0,
                           channel_multiplier=0)

            # ---- e0 int24 fixed-point reconstruction -> acc (exact) ----
            acc = accp.tile([128, T * D], f32)
            with tc.tile_pool(name="init", bufs=1) as initp:
                e0hs = initp.tile([128, T, D], i16)
                nc.sync.dma_start(out=e0hs[:],
                                  in_=e0hi[:].rearrange("(t p) d -> p t d", p=128))
                e0ls = initp.tile([128, T, D], u8)
                nc.sync.dma_start(out=e0ls[:],
                                  in_=e0lo[:].rearrange("(t p) d -> p t d", p=128))
                tmp = initp.tile([128, T * D], f32)
                nc.scalar.copy(out=acc[:], in_=e0hs[:].rearrange("p t d -> p (t d)"))
                nc.vector.tensor_scalar_mul(out=acc[:], in0=acc[:], scalar1=256.0)
                nc.scalar.copy(out=tmp[:], in_=e0ls[:].rearrange("p t d -> p (t d)"))
                nc.vector.tensor_add(out=acc[:], in0=acc[:], in1=tmp[:])
                nc.vector.tensor_scalar_mul(out=acc[:], in0=acc[:],
                                            scalar1=E0_SCALE)

            tb0 = dram.tile([N_PAD, D], f32, tag="tb0", addr_space="Shared")
            tb1 = dram.tile([N_PAD, D], f32, tag="tb1", addr_space="Shared")
            tb2 = dram.tile([N_PAD, D], f32, tag="tb2", addr_space="Shared")
            sh1 = dram.tile([R, D], f32, tag="sh1")
            sh2 = dram.tile([R, D], f32, tag="sh2")
            accd = dram.tile([R, D], f32, tag="accd")
            accf = dram.tile([N_PAD, D], f32, tag="accf", addr_space="Shared")
            ues = dram.tile([2 * BATCH + 128, D], f32, tag="ues")  # +dump rows

            # full node table from per-core shards.
            # Collectives cannot read IO tensors -> stage through sh0.
            sh0 = dram.tile([R, D], f32, tag="sh0")
            nc.sync.dma_start(out=sh0[:].rearrange("(t p) d -> p t d", p=128),
                              in_=acc[:].rearrange("p (t d) -> p t d", d=D))
            nc.gpsimd.collective_compute("AllGather", mybir.AluOpType.bypass,
                                         replica_groups=RG,
                                         ins=[sh0[:]], outs=[tb0[:]])
            # zero the epilogue scatter target early (overlaps with layers)
            zt = constp.tile([128, 2 * BATCH // 128, D], f32)
            nc.vector.memset(zt[:], 0)
            nc.sync.dma_start(
                out=ues[:][0:2 * BATCH, :].rearrange("(c p) d -> p c d", p=128),
                in_=zt[:])
            egs = constp.tile([128, G // 16], i16)
            ess = constp.tile([128, G // 16], i16)
            for a in range(8):
                nc.sync.dma_start(out=egs[16 * a:16 * (a + 1), :], in_=egi[:])
                nc.sync.dma_start(out=ess[16 * a:16 * (a + 1), :], in_=esi[:])

            tables = [tb0, tb1, tb2]
            shards = [sh1, sh2]

            with tc.tile_pool(name="meta", bufs=2) as metap, \
                 tc.tile_pool(name="gp", bufs=3) as gp, \
                 tc.tile_pool(name="sp", bufs=2) as sp, \
                 tc.tile_pool(name="ob", bufs=4) as obp:
                for layer in range(3):
                    src = tables[layer][:]
                    for s in range(T // STRIP):
                        cols = STRIP * LT
                        c0s = s * cols
                        ixs = metap.tile([128, cols * 8], i16, tag="ixs")
                        for a in range(8):
                            nc.sync.dma_start(
                                out=ixs[16 * a:16 * (a + 1), :],
                                in_=idxc[:, c0s * 8:(c0s + cols) * 8])
                        vls16 = metap.tile([128, cols], u16, tag="vls16")
                        nc.sync.dma_start(out=vls16[:], in_=valv[:, c0s:c0s + cols])
                        vls = metap.tile([128, cols], f32, tag="vls")
                        nc.scalar.copy(out=vls[:], in_=vls16[:])
                        nc.vector.tensor_scalar_mul(out=vls[:], in0=vls[:],
                                                    scalar1=EV_SCALE)
                        rls8 = metap.tile([128, cols], u8, tag="rls8")
                        nc.sync.dma_start(out=rls8[:], in_=rlv[:, c0s:c0s + cols])
                        rls = metap.tile([128, cols], i16, tag="rls")
                        nc.scalar.copy(out=rls[:], in_=rls8[:])
                        for tt in range(STRIP):
                            t = s * STRIP + tt
                            ps = psp.tile([128, D], f32)
                            S = sp.tile([128, LT, 128], f32, tag="S")
                            nc.vector.tensor_tensor(
                                out=S[:],
                                in0=rls[:, tt * LT:(tt + 1) * LT].to_broadcast([128, LT, 128]),
                                in1=iota[:].to_broadcast([128, LT, 128]),
                                op=mybir.AluOpType.is_equal)
                            for b in range(NB):
                                Lb = L[b]
                                if Lb == 0:
                                    continue
                                g = gp.tile([128, Lb, D], f32, tag=f"g{b}")
                                ib = (tt * LT + segc[b]) * 8
                                nc.gpsimd.dma_gather(
                                    g[:], src[BANK_BASE[b]:, :],
                                    ixs[:, ib:ib + Lb * 8], Lb * 128, Lb * 128,
                                    D, single_packet=False)
                                vb = tt * LT + segc[b]
                                nc.vector.tensor_tensor(
                                    out=g[:],
                                    in0=vls[:, vb:vb + Lb].to_broadcast([128, Lb, D]),
                                    in1=g[:],
                                    op=mybir.AluOpType.mult)
                                for k in range(Lb):
                                    kk = segc[b] + k
                                    nc.tensor.matmul(
                                        out=ps[:], lhsT=S[:, kk, :], rhs=g[:, k, :],
                                        start=(kk == 0), stop=(kk == LT - 1))
                            nc.vector.tensor_add(out=acc[:, t * D:(t + 1) * D],
                                                 in0=acc[:, t * D:(t + 1) * D],
                                                 in1=ps[:])
                            if layer < 2:
                                ob = obp.tile([128, D], f32, tag="ob")
                                nc.scalar.copy(out=ob[:], in_=ps[:])
                                nc.sync.dma_start(
                                    out=shards[layer][:].rearrange(
                                        "(t p) d -> p t d", p=128)[:, t, :],
                                    in_=ob[:])
                        del ixs, vls16, vls, rls8, rls
                    if layer < 2:
                        nc.gpsimd.collective_compute(
                            "AllGather", mybir.AluOpType.bypass,
                            replica_groups=RG,
                            ins=[shards[layer][:]], outs=[tables[layer + 1][:]])

            # ---- epilogue: gamma[s] = (acc[u_s] . acc[N_USERS+i_s]) / 16 ----
            nc.sync.dma_start(out=accd[:].rearrange("(t p) d -> p t d", p=128),
                              in_=acc[:].rearrange("p (t d) -> p t d", d=D))
            nc.gpsimd.collective_compute("AllGather", mybir.AluOpType.bypass,
                                         replica_groups=RG,
                                         ins=[accd[:]], outs=[accf[:]])
            with tc.tile_pool(name="ep", bufs=1) as epp:
                half = BATCH // 128
                goff = 0
                for b in range(NB):
                    cap = EPC[b]
                    if cap == 0:
                        continue
                    gb = epp.tile([128, cap // 128, D], f32, tag=f"eg{b}")
                    nc.gpsimd.dma_gather(
                        gb[:], accf[:][BANK_BASE[b]:, :],
                        egs[:, goff // 16:(goff + cap) // 16], cap, cap,
                        D, single_packet=False)
                    nc.gpsimd.dma_scatter_add(
                        ues[:], gb[:],
                        ess[:, goff // 16:(goff + cap) // 16], cap, cap,
                        D, single_packet=False)
                    goff += cap
                ue_sb = epp.tile([128, 2 * BATCH // 128, D], f32)
                nc.sync.dma_start(
                    out=ue_sb[:],
                    in_=ues[:][0:2 * BATCH, :].rearrange("(c p) d -> p c d", p=128))
                prod = epp.tile([128, half, D], f32)
                nc.vector.tensor_tensor(out=prod[:], in0=ue_sb[:, 0:half, :],
                                        in1=ue_sb[:, half:2 * half, :],
                                        op=mybir.AluOpType.mult)
                gm = epp.tile([128, half], f32)
                nc.vector.tensor_reduce(out=gm[:], in_=prod[:],
                                        axis=mybir.AxisListType.X,
                                        op=mybir.AluOpType.add)
                gms = epp.tile([128, half], f32)
                nc.vector.tensor_scalar_mul(out=gms[:], in0=gm[:],
                                            scalar1=1.0 / 16.0)
                nc.sync.dma_start(out=outg[:].rearrange("(c p) -> p c", p=128),
                                  in_=gms[:])
    nc.compile()
    return nc


def _encode_e0(user_emb, item_emb):
    """e0 -> int24 fixed-point planes (hi int16, lo uint8), scale 2^-23.

    Reconstruction (hi*256 + lo) * 2^-23 is exact in f32.
    """
    e0 = np.zeros((N_PAD, D), dtype=np.float64)
    e0[:N_USERS] = np.asarray(user_emb, dtype=np.float64)
    e0[N_USERS:N] = np.asarray(item_emb, dtype=np.float64)
    v = np.clip(np.round(e0 / E0_SCALE), -(2 ** 23), 2 ** 23 - 1).astype(np.int32)
    hi = (v >> 8).astype(np.int16)
    lo = (v & 255).astype(np.uint8)
    return hi, lo


def kernel(user_emb, item_emb, edge_row, edge_col, edge_val, users, items):
    _configure_jax_cache()
    from concourse.bass_utils import run_bass_kernel_spmd

    e0hi, e0lo = _encode_e0(user_emb, item_emb)
    L, idxc, valv, rlv = _preprocess(edge_row, edge_col, edge_val)
    caps, gw, sw = _ep_meta(users, items)
    key = (L, caps)
    if key not in _compiled:
        _compiled[key] = _build(L, caps)
    nc = _compiled[key]

    in_maps = []
    for c in range(NCORES):
        in_maps.append({
            "e0hi": e0hi[c * R:(c + 1) * R],
            "e0lo": e0lo[c * R:(c + 1) * R],
            "idxc": idxc[c],
            "valv": valv[c],
            "rlv": rlv[c],
            "egi": gw,
            "esi": sw,
        })
    res = run_bass_kernel_spmd(nc, in_maps, core_ids=list(range(NCORES)))
    return np.asarray(res.results[0]["outg"], dtype=np.float32)


# revision 4
# speedup vs baseline: 2.7765x; 1.2909x over previous
"""LightGCN 3-layer propagation + batch dot on 8 Trainium2 NeuronCores.

Transfer-diet + dispatch-diet revision. Per-call device invocation cost
under the axon tunnel decomposes as: MLIR lower (~0.3s), BIR->NEFF walrus
recompile (~2.0s without caching), H2D at ~60-100MB/s, device exec
(~0.2-0.4s), D2H (tiny). This version attacks all of them:
  (a) JAX persistent compilation cache (configured at kernel() time) so
      repeat calls skip the ~2s walrus/neuronxcc recompile entirely,
  (b) nc.to_json_bytes memoized on the instance after compile() - the
      bass_exec MLIR lowering re-serializes the 26MB BIR on every call
      (~0.21s) and the module is frozen after compile, so cache it,
  (c) e0 shipped as int16 fixed-point (scale 2^-15; range +-1 covers the
      0.1-sigma gaussian data with no clipping) for propagation, PLUS an
      exact-direct-term correction: the f32 residual (e0 - q(e0))/4 at
      each core's 1024 batch rows, shipped fp16 scaled by 2^12 (values
      ~1e-6 are fp16-subnormal unscaled). Host-measured max rel err
      6.8e-3 vs the f32 reference (2.9x margin; e0 fp16 FAILS at 2.1e-1,
      bare int16 without the residual FAILS at 1.3e-1),
  (d) edge vals shipped as uint16 fixed-point (scale 2^-22, exact f32
      reconstruct; max val 0.01*2^22 = 41943 < 65536),
  (e) the batch-dot epilogue is sharded across cores (core c computes
      gamma[512c:512c+512]) instead of 8x-redundant: per-core banked
      gather of its 1024 acc rows from the AllGathered table, residual
      add, dot, outg [512] per core assembled on host.
All device arithmetic stays f32 (gather tables, products, PSUM).

Per layer (unchanged): row-partitioned nodes, per (128-row tile,
col-bank) fixed-capacity edge chunks, dma_gather of source embeddings
(col-sorted within chunks), val multiply, one-hot f32 matmuls
segment-summing into f32 PSUM, AllGather of the next layer table.
DGE note: num_idxs_reg must equal the count of non-negative indices, so
all index streams are padded with valid indices (row 0 for gathers, dump
rows past the real slots for the epilogue scatter).
"""
import numpy as np

N_USERS = 100000
N_ITEMS = 50000
N = N_USERS + N_ITEMS        # 150000
D = 64
NCORES = 8
N_PAD = 150528               # 8 * 18816
R = N_PAD // NCORES          # 18816 rows per core
T = R // 128                 # 147 row-tiles per core
STRIP = 21                   # tiles per metadata strip (147 = 7*21)
NB = 5                       # 32768-row col banks
BANK_BASE = (0, 32768, 65536, 98304, 131072)
BATCH = 4096
SL = BATCH // NCORES         # 512 gamma slots per core
E0_SCALE = float(2.0 ** -15)  # e0 int16 fixed-point scale
EV_SCALE = float(2.0 ** -22)  # edge_val uint16 fixed-point scale
RES_SCALE = float(2.0 ** 12)  # residual pre-scale (keeps fp16 normal)

_compiled = {}
_cache_configured = False


def _configure_jax_cache():
    """Persistent compilation cache: repeat calls (and repeat processes)
    skip the ~2s BIR->NEFF walrus recompile that run_bass_via_pjrt's
    fresh-jit-per-call path otherwise re-runs every invocation."""
    global _cache_configured
    if _cache_configured:
        return
    import jax
    jax.config.update("jax_compilation_cache_dir", "/tmp/jax_bass_cache")
    jax.config.update("jax_persistent_cache_min_compile_time_secs", 0.0)
    jax.config.update("jax_persistent_cache_min_entry_size_bytes", 0)
    _cache_configured = True


def _preprocess(edge_row, edge_col, edge_val):
    """Sort/pad edges into per-core fixed-capacity (tile, bank) chunks.

    Returns (L, idxc [8,16,C*8] i16, valv [8,128,C] u16, rlv [8,128,C] u8)
    where L = per-bank chunk capacities (in 128-edge units) and C = T*sum(L).
    Within each chunk edges are sorted by column index for gather locality.
    Vals are uint16 fixed-point (v * 2^22 rounded; exact f32 reconstruct).
    """
    er = np.asarray(edge_row).astype(np.int64)
    ec = np.asarray(edge_col).astype(np.int64)
    ev = np.asarray(edge_val).astype(np.float64)

    owner = er // R
    rrem = er - owner * R
    tloc = rrem >> 7
    rl = (rrem & 127).astype(np.uint8)
    bank = ec >> 15
    cidx = (ec & 32767).astype(np.int16)

    seg = (owner * T + tloc) * NB + bank
    nseg = NCORES * T * NB
    counts = np.bincount(seg, minlength=nseg)
    cmax = counts.reshape(NCORES, T, NB).max(axis=(0, 1))
    L = tuple(int(-(-int(c) // 128)) for c in cmax)     # ceil/128
    LT = sum(L)
    C = T * LT

    order = np.argsort(seg * 32768 + cidx.astype(np.int64))
    sseg = seg[order]
    starts = np.concatenate([[0], np.cumsum(counts)[:-1]])
    rank = np.arange(len(order)) - starts[sseg]

    segL = np.concatenate([[0], np.cumsum(L)[:-1]])
    core_o, tloc_o, bank_o = owner[order], tloc[order], bank[order]
    pos = (tloc_o * LT + segL[bank_o]) * 128 + rank     # within-core edge slot

    evq = np.clip(np.round(ev / EV_SCALE), 0, 65535).astype(np.uint16)

    E_cap = C * 128
    # pad slots: idx 0 (valid row, gathers garbage), val 0, rl 0 -> adds 0
    cidx_a = np.zeros((NCORES, E_cap), dtype=np.int16)
    val_a = np.zeros((NCORES, E_cap), dtype=np.uint16)
    rl_a = np.zeros((NCORES, E_cap), dtype=np.uint8)
    cidx_a[core_o, pos] = cidx[order]
    val_a[core_o, pos] = evq[order]
    rl_a[core_o, pos] = rl[order]

    idxc = cidx_a.reshape(NCORES, C * 8, 16).transpose(0, 2, 1).copy()  # [8,16,C*8]
    valv = val_a.reshape(NCORES, C, 128).transpose(0, 2, 1).copy()      # [8,128,C]
    rlv = rl_a.reshape(NCORES, C, 128).transpose(0, 2, 1).copy()        # [8,128,C]
    return L, idxc, valv, rlv


def _ep_meta(users, items):
    """Per-core banked gather + slot-scatter indices for the sharded
    batch-dot epilogue. Core c computes gamma[SL*c : SL*(c+1)]; its local
    slot s in [0,SL) is user users[SL*c+s], slot SL+s is item items[...].

    Returns (caps, gw [8,16,G/16] i16, sw [8,16,G/16] i16) with
    G = sum(caps) shared across cores (SPMD static shapes; caps = per-bank
    max count over cores, rounded to 128). All indices are valid
    (num_idxs_reg must equal the count of non-negative indices): gather
    pads hit row 0 of the bank, scatter pads land in the 128 dump rows
    past the 2*SL real slots.
    """
    users = np.asarray(users).astype(np.int64)
    items = np.asarray(items).astype(np.int64)
    core_rows = []
    for c in range(NCORES):
        u = users[SL * c:SL * (c + 1)]
        i = N_USERS + items[SL * c:SL * (c + 1)]
        core_rows.append(np.concatenate([u, i]))        # local slots 0..2*SL-1
    counts = np.zeros((NCORES, NB), dtype=np.int64)
    for c in range(NCORES):
        counts[c] = np.bincount(core_rows[c] >> 15, minlength=NB)
    caps = tuple(int(-(-int(counts[:, b].max()) // 128) * 128) for b in range(NB))
    G = sum(caps)
    gw = np.zeros((NCORES, 16, G // 16), dtype=np.int16)
    sw = np.zeros((NCORES, 16, G // 16), dtype=np.int16)
    for c in range(NCORES):
        rows = core_rows[c]
        bank = rows >> 15
        cidx = (rows & 32767).astype(np.int16)
        order = np.argsort(bank, kind="stable")
        gidx = np.zeros(G, dtype=np.int16)
        sidx = (2 * SL + (np.arange(G) & 127)).astype(np.int16)  # dump rows
        off = 0
        p = 0
        for b in range(NB):
            cnt = int(counts[c, b])
            sel = order[p:p + cnt]
            p += cnt
            gidx[off:off + cnt] = cidx[sel]
            sidx[off:off + cnt] = sel.astype(np.int16)  # local slot id
            off += caps[b]
        gw[c] = gidx.reshape(G // 16, 16).T
        sw[c] = sidx.reshape(G // 16, 16).T
    return caps, gw, sw


def _build(L, EPC):
    import concourse.bacc as bacc
    import concourse.mybir as mybir
    import concourse.tile as tile
    from concourse.library_config import mlp

    LT = sum(L)
    C = T * LT
    G = sum(EPC)
    f32 = mybir.dt.float32
    f16 = mybir.dt.float16
    i16 = mybir.dt.int16
    u16 = mybir.dt.uint16
    u8 = mybir.dt.uint8

    nc = bacc.Bacc("TRN2", target_bir_lowering=False, debug=False,
                   num_devices=NCORES)
    e0q = nc.dram_tensor("e0q", [R, D], i16, kind="ExternalInput")
    eres = nc.dram_tensor("eres", [2 * SL, D], f16, kind="ExternalInput")
    idxc = nc.dram_tensor("idxc", [16, C * 8], i16, kind="ExternalInput")
    valv = nc.dram_tensor("valv", [128, C], u16, kind="ExternalInput")
    rlv = nc.dram_tensor("rlv", [128, C], u8, kind="ExternalInput")
    egi = nc.dram_tensor("egi", [16, G // 16], i16, kind="ExternalInput")
    esi = nc.dram_tensor("esi", [16, G // 16], i16, kind="ExternalInput")
    outg = nc.dram_tensor("outg", [SL], f32, kind="ExternalOutput")

    segc = [0]
    for x in L[:-1]:
        segc.append(segc[-1] + x)
    RG = [list(range(NCORES))]

    with tile.TileContext(nc, num_cores=NCORES) as tc:
        with tc.tile_pool(name="const", bufs=1) as constp, \
             tc.tile_pool(name="accp", bufs=1) as accp, \
             tc.tile_pool(name="psum", bufs=4, space="PSUM") as psp, \
             tc.tile_pool(name="dram", bufs=1, space="DRAM") as dram:
            nc.gpsimd.load_library(mlp)
            iota = constp.tile([128, 1, 128], i16)
            nc.gpsimd.iota(iota[:, 0, :], pattern=[[1, 128]], base=0,
                           channel_multiplier=0)

            # ---- e0 int16 fixed-point reconstruction -> acc ----
            acc = accp.tile([128, T * D], f32)
            with tc.tile_pool(name="init", bufs=1) as initp:
                e0qs = initp.tile([128, T, D], i16)
                nc.sync.dma_start(out=e0qs[:],
                                  in_=e0q[:].rearrange("(t p) d -> p t d", p=128))
                nc.scalar.copy(out=acc[:], in_=e0qs[:].rearrange("p t d -> p (t d)"))
                nc.vector.tensor_scalar_mul(out=acc[:], in0=acc[:],
                                            scalar1=E0_SCALE)

            tb0 = dram.tile([N_PAD, D], f32, tag="tb0", addr_space="Shared")
            tb1 = dram.tile([N_PAD, D], f32, tag="tb1", addr_space="Shared")
            tb2 = dram.tile([N_PAD, D], f32, tag="tb2", addr_space="Shared")
            sh1 = dram.tile([R, D], f32, tag="sh1")
            sh2 = dram.tile([R, D], f32, tag="sh2")
            accd = dram.tile([R, D], f32, tag="accd")
            accf = dram.tile([N_PAD, D], f32, tag="accf", addr_space="Shared")
            ues = dram.tile([2 * SL + 128, D], f32, tag="ues")  # +dump rows

            # full node table from per-core shards.
            # Collectives cannot read IO tensors -> stage through sh0.
            sh0 = dram.tile([R, D], f32, tag="sh0")
            nc.sync.dma_start(out=sh0[:].rearrange("(t p) d -> p t d", p=128),
                              in_=acc[:].rearrange("p (t d) -> p t d", d=D))
            nc.gpsimd.collective_compute("AllGather", mybir.AluOpType.bypass,
                                         replica_groups=RG,
                                         ins=[sh0[:]], outs=[tb0[:]])
            # zero the epilogue scatter target early (overlaps with layers)
            zt = constp.tile([128, (2 * SL + 128) // 128, D], f32)
            nc.vector.memset(zt[:], 0)
            nc.sync.dma_start(
                out=ues[:].rearrange("(c p) d -> p c d", p=128),
                in_=zt[:])
            egs = constp.tile([128, G // 16], i16)
            ess = constp.tile([128, G // 16], i16)
            for a in range(8):
                nc.sync.dma_start(out=egs[16 * a:16 * (a + 1), :], in_=egi[:])
                nc.sync.dma_start(out=ess[16 * a:16 * (a + 1), :], in_=esi[:])

            tables = [tb0, tb1, tb2]
            shards = [sh1, sh2]

            with tc.tile_pool(name="meta", bufs=2) as metap, \
                 tc.tile_pool(name="gp", bufs=3) as gp, \
                 tc.tile_pool(name="sp", bufs=2) as sp, \
                 tc.tile_pool(name="ob", bufs=4) as obp:
                for layer in range(3):
                    src = tables[layer][:]
                    for s in range(T // STRIP):
                        cols = STRIP * LT
                        c0s = s * cols
                        ixs = metap.tile([128, cols * 8], i16, tag="ixs")
                        for a in range(8):
                            nc.sync.dma_start(
                                out=ixs[16 * a:16 * (a + 1), :],
                                in_=idxc[:, c0s * 8:(c0s + cols) * 8])
                        vls16 = metap.tile([128, cols], u16, tag="vls16")
                        nc.sync.dma_start(out=vls16[:], in_=valv[:, c0s:c0s + cols])
                        vls = metap.tile([128, cols], f32, tag="vls")
                        nc.scalar.copy(out=vls[:], in_=vls16[:])
                        nc.vector.tensor_scalar_mul(out=vls[:], in0=vls[:],
                                                    scalar1=EV_SCALE)
                        rls8 = metap.tile([128, cols], u8, tag="rls8")
                        nc.sync.dma_start(out=rls8[:], in_=rlv[:, c0s:c0s + cols])
                        rls = metap.tile([128, cols], i16, tag="rls")
                        nc.scalar.copy(out=rls[:], in_=rls8[:])
                        for tt in range(STRIP):
                            t = s * STRIP + tt
                            ps = psp.tile([128, D], f32)
                            S = sp.tile([128, LT, 128], f32, tag="S")
                            nc.vector.tensor_tensor(
                                out=S[:],
                                in0=rls[:, tt * LT:(tt + 1) * LT].to_broadcast([128, LT, 128]),
                                in1=iota[:].to_broadcast([128, LT, 128]),
                                op=mybir.AluOpType.is_equal)
                            for b in range(NB):
                                Lb = L[b]
                                if Lb == 0:
                                    continue
                                g = gp.tile([128, Lb, D], f32, tag=f"g{b}")
                                ib = (tt * LT + segc[b]) * 8
                                nc.gpsimd.dma_gather(
                                    g[:], src[BANK_BASE[b]:, :],
                                    ixs[:, ib:ib + Lb * 8], Lb * 128, Lb * 128,
                                    D, single_packet=False)
                                vb = tt * LT + segc[b]
                                nc.vector.tensor_tensor(
                                    out=g[:],
                                    in0=vls[:, vb:vb + Lb].to_broadcast([128, Lb, D]),
                                    in1=g[:],
                                    op=mybir.AluOpType.mult)
                                for k in range(Lb):
                                    kk = segc[b] + k
                                    nc.tensor.matmul(
                                        out=ps[:], lhsT=S[:, kk, :], rhs=g[:, k, :],
                                        start=(kk == 0), stop=(kk == LT - 1))
                            nc.vector.tensor_add(out=acc[:, t * D:(t + 1) * D],
                                                 in0=acc[:, t * D:(t + 1) * D],
                                                 in1=ps[:])
                            if layer < 2:
                                ob = obp.tile([128, D], f32, tag="ob")
                                nc.scalar.copy(out=ob[:], in_=ps[:])
                                nc.sync.dma_start(
                                    out=shards[layer][:].rearrange(
                                        "(t p) d -> p t d", p=128)[:, t, :],
                                    in_=ob[:])
                        del ixs, vls16, vls, rls8, rls
                    if layer < 2:
                        nc.gpsimd.collective_compute(
                            "AllGather", mybir.AluOpType.bypass,
                            replica_groups=RG,
                            ins=[shards[layer][:]], outs=[tables[layer + 1][:]])

            # ---- epilogue (sharded): gamma[SL*c + s]
            #      = ((acc[u_s] + res_u) . (acc[i_s] + res_i)) / 16 ----
            nc.sync.dma_start(out=accd[:].rearrange("(t p) d -> p t d", p=128),
                              in_=acc[:].rearrange("p (t d) -> p t d", d=D))
            nc.gpsimd.collective_compute("AllGather", mybir.AluOpType.bypass,
                                         replica_groups=RG,
                                         ins=[accd[:]], outs=[accf[:]])
            with tc.tile_pool(name="ep", bufs=1) as epp:
                half = SL // 128                     # 4
                goff = 0
                for b in range(NB):
                    cap = EPC[b]
                    if cap == 0:
                        continue
                    gb = epp.tile([128, cap // 128, D], f32, tag=f"eg{b}")
                    nc.gpsimd.dma_gather(
                        gb[:], accf[:][BANK_BASE[b]:, :],
                        egs[:, goff // 16:(goff + cap) // 16], cap, cap,
                        D, single_packet=False)
                    nc.gpsimd.dma_scatter_add(
                        ues[:], gb[:],
                        ess[:, goff // 16:(goff + cap) // 16], cap, cap,
                        D, single_packet=False)
                    goff += cap
                ue_sb = epp.tile([128, 2 * half, D], f32)
                nc.sync.dma_start(
                    out=ue_sb[:],
                    in_=ues[:][0:2 * SL, :].rearrange("(c p) d -> p c d", p=128))
                # exact-direct-term residual: ue += eres * 2^-12
                res16 = epp.tile([128, 2 * half, D], f16)
                nc.sync.dma_start(
                    out=res16[:],
                    in_=eres[:].rearrange("(c p) d -> p c d", p=128))
                resf = epp.tile([128, 2 * half, D], f32)
                nc.scalar.copy(out=resf[:], in_=res16[:])
                nc.vector.tensor_scalar_mul(out=resf[:], in0=resf[:],
                                            scalar1=1.0 / RES_SCALE)
                nc.vector.tensor_add(out=ue_sb[:], in0=ue_sb[:], in1=resf[:])
                prod = epp.tile([128, half, D], f32)
                nc.vector.tensor_tensor(out=prod[:], in0=ue_sb[:, 0:half, :],
                                        in1=ue_sb[:, half:2 * half, :],
                                        op=mybir.AluOpType.mult)
                gm = epp.tile([128, half], f32)
                nc.vector.tensor_reduce(out=gm[:], in_=prod[:],
                                        axis=mybir.AxisListType.X,
                                        op=mybir.AluOpType.add)
                gms = epp.tile([128, half], f32)
                nc.vector.tensor_scalar_mul(out=gms[:], in0=gm[:],
                                            scalar1=1.0 / 16.0)
                nc.sync.dma_start(out=outg[:].rearrange("(c p) -> p c", p=128),
                                  in_=gms[:])
    nc.compile()
    # The bass_exec MLIR lowering calls nc.to_json_bytes() on EVERY device
    # call (~0.21s for this 26MB module). The module is frozen after
    # compile(), so serialize once and pin the bytes on the instance.
    _json_bytes = nc.to_json_bytes()
    nc.to_json_bytes = lambda: _json_bytes
    return nc


def _encode_e0(user_emb, item_emb):
    """e0 -> int16 fixed-point (scale 2^-15; +-1 range, no clipping for
    the 0.1-sigma gaussian data). Returns (e0q [N_PAD,D] i16,
    e0dec [N_PAD,D] f64 decoded values for residual computation)."""
    e0 = np.zeros((N_PAD, D), dtype=np.float64)
    e0[:N_USERS] = np.asarray(user_emb, dtype=np.float64)
    e0[N_USERS:N] = np.asarray(item_emb, dtype=np.float64)
    v = np.clip(np.round(e0 / E0_SCALE), -32768, 32767).astype(np.int16)
    return v, v.astype(np.float64) * E0_SCALE, e0


def _encode_resid(e0, e0dec, users, items):
    """Per-core direct-term residuals: ((e0 - q(e0)) * 2^12) as fp16
    [8, 2*SL, D], rows in local slot order (SL users then SL items).
    NOT pre-divided by 4: the gathered acc rows are the un-divided
    e0+l1+l2+l3 sum (the /4 per vector is folded into the final 1/16)."""
    users = np.asarray(users).astype(np.int64)
    items = np.asarray(items).astype(np.int64)
    out = np.zeros((NCORES, 2 * SL, D), dtype=np.float16)
    resid = (e0 - e0dec) * RES_SCALE
    for c in range(NCORES):
        rows = np.concatenate([
            users[SL * c:SL * (c + 1)],
            N_USERS + items[SL * c:SL * (c + 1)],
        ])
        out[c] = resid[rows].astype(np.float16)
    return out


def kernel(user_emb, item_emb, edge_row, edge_col, edge_val, users, items):
    _configure_jax_cache()
    from concourse.bass_utils import run_bass_kernel_spmd

    e0q, e0dec, e0 = _encode_e0(user_emb, item_emb)
    eres = _encode_resid(e0, e0dec, users, items)
    L, idxc, valv, rlv = _preprocess(edge_row, edge_col, edge_val)
    caps, gw, sw = _ep_meta(users, items)
    key = (L, caps)
    if key not in _compiled:
        _compiled[key] = _build(L, caps)
    nc = _compiled[key]

    in_maps = []
    for c in range(NCORES):
        in_maps.append({
            "e0q": e0q[c * R:(c + 1) * R],
            "eres": eres[c],
            "idxc": idxc[c],
            "valv": valv[c],
            "rlv": rlv[c],
            "egi": gw[c],
            "esi": sw[c],
        })
    res = run_bass_kernel_spmd(nc, in_maps, core_ids=list(range(NCORES)))
    return np.concatenate(
        [np.asarray(res.results[c]["outg"], dtype=np.float32)
         for c in range(NCORES)])
